# revision 6
# baseline (speedup 1.0000x reference)
"""Trainium2 Bass kernel for 2-layer heterogeneous GraphConv + MLP head.

Strategy (8 NeuronCores, SPMD):
  - Nodes (and their incoming edges) are partitioned by dst across the 8
    cores (12500 nodes each). Each core holds a full copy of the node
    features in its HBM, gathers h[src] for its edges with dma_gather
    (int16 indices, relative to one of 4 source sections of 25000 rows),
    and scatter-adds messages into per-chunk PSUM accumulators using
    one-hot matmuls on the TensorEngine (degree normalization folded into
    the one-hot values).  Per-etype GraphConv outputs are combined as
    relu(x)/3 sums; between the two conv layers the new features are
    exchanged with an AllGather collective.  The linear head (two matmuls,
    no nonlinearity between) is folded into a single [128,32] matmul and
    applied per chunk in the transposed orientation.
"""

import sys

sys.path.insert(0, "/opt/trn_rl_repo")

import numpy as np
import ml_dtypes

import concourse.bass as bass
import concourse.bacc as bacc
import concourse.mybir as mybir
import concourse.tile as tile
from concourse.masks import make_identity
from concourse.bass_utils import run_bass_kernel_spmd
from concourse.library_config import mlp

N_NODES = 100000
N_ETYPES = 3
N_EDGES = 1600000
CONV_LAYERS = 2
D_IN, D_HID, D_OUT = 128, 256, 32

N_CORES = 8
NPC = N_NODES // N_CORES          # nodes per core = 12500
N_SEC = 4                         # int16-addressable source sections
SEC = N_NODES // N_SEC            # 25000 rows per section
CH = 512                          # dst chunk width (one PSUM bank)
NCHUNK = (NPC + CH - 1) // CH     # 25 chunks (last = 212 nodes)
RANGE = 2                         # chunks per gather-call range
P = 128

BF16 = mybir.dt.bfloat16
F32 = mybir.dt.float32


def _prep(blocks, edge_src, edge_dst, conv_W, conv_b, W1, b1, W2, b2):
    """Host-side index preprocessing. Returns per-core input maps plus the
    compile-time structure (slot capacities) shared by all cores."""
    deg = np.stack([np.bincount(edge_dst[e], minlength=N_NODES)
                    for e in range(N_ETYPES)]).astype(np.float32)
    recip3 = 1.0 / (3.0 * np.maximum(deg, 1.0))          # [E_T, N]

    # per (etype): core/section/chunk of every edge
    counts = np.zeros((N_CORES, N_ETYPES, N_SEC, NCHUNK), np.int64)
    per_e = []
    for e in range(N_ETYPES):
        src = edge_src[e].astype(np.int64)
        dst = edge_dst[e].astype(np.int64)
        core = dst // NPC
        sec = src // SEC
        chunk = (dst % NPC) // CH
        order = np.lexsort((src, chunk, sec, core))
        src, dst, core, sec, chunk = (a[order] for a in (src, dst, core, sec, chunk))
        gid = ((core * N_SEC + sec) * NCHUNK) + chunk     # group id within etype
        cnt = np.bincount(gid, minlength=N_CORES * N_SEC * NCHUNK)
        counts[:, e] = cnt.reshape(N_CORES, N_SEC, NCHUNK)
        per_e.append((src, dst, core, sec, chunk, gid))

    # capacities: identical across cores (SPMD), from actual data
    caps = counts.max(axis=0)                             # [E_T, N_SEC, NCHUNK]
    caps = ((caps + P - 1) // P) * P                      # tile aligned
    total_slots = int(caps.sum())

    # slot offsets: layout order (e, s, c)
    offs = np.zeros((N_ETYPES, N_SEC, NCHUNK), np.int64)
    flat = caps.reshape(-1)
    offs.reshape(-1)[:] = np.concatenate(([0], np.cumsum(flat)[:-1]))

    n_tiles = total_slots // P

    in_maps = []
    for c in range(N_CORES):
        gidx = np.zeros(total_slots, np.int16)
        dstc = np.full(total_slots, -1.0, np.float32)
        wv = np.zeros(total_slots, np.float32)
        for e in range(N_ETYPES):
            src, dst, core, sec, chunk, gid = per_e[e]
            m = core == c
            s_, d_, se_, ch_, g_ = src[m], dst[m], sec[m], chunk[m], gid[m]
            g_ -= c * N_SEC * NCHUNK
            # rank within group (edges already sorted by group then src)
            grp_start = np.concatenate(([0], np.cumsum(np.bincount(
                g_, minlength=N_SEC * NCHUNK))[:-1]))
            rank = np.arange(len(g_)) - grp_start[g_]
            slot = offs[e].reshape(-1)[g_] + rank
            gidx[slot] = (s_ % SEC).astype(np.int16)
            dstc[slot] = (d_ % NPC) % CH
            wv[slot] = recip3[e, d_]
            # pads: point at row 0 of the section (valid row, dstc=-1 -> A row 0)
        # wrap idx into [16, total/16] then replicate to 128 partitions
        w16 = gidx.reshape(total_slots // 16, 16).T.copy()
        idx_rep = np.tile(w16, (8, 1))                    # [128, total/16]
        dstc_t = dstc.reshape(n_tiles, P).T.copy()        # [128, n_tiles]
        wv_t = wv.reshape(n_tiles, P).T.copy()

        im = {
            "blocks16": np.ascontiguousarray(blocks.astype(ml_dtypes.bfloat16)),
            "gidx": idx_rep,
            "dstc": dstc_t,
            "wv": wv_t,
        }
        in_maps.append(im)

    shared = {
        "iota": np.tile(np.arange(CH, dtype=np.float32), (P, 1)),
        "convW16": np.ascontiguousarray(
            conv_W.astype(ml_dtypes.bfloat16)),              # [L,E,128,128]
        "convb3": np.ascontiguousarray(
            (conv_b / 3.0).astype(np.float32).reshape(CONV_LAYERS, N_ETYPES, D_IN, 1)),
        "W12": np.ascontiguousarray((W1.astype(np.float64) @ W2.astype(np.float64)).astype(np.float32)),
        "b12": np.ascontiguousarray(
            (b1.astype(np.float64) @ W2.astype(np.float64) + b2).astype(np.float32).reshape(D_OUT, 1)),
    }
    for im in in_maps:
        im.update(shared)
    return in_maps, caps, offs, total_slots


def _ranges():
    r = []
    c = 0
    while c < NCHUNK:
        r.append(list(range(c, min(c + RANGE, NCHUNK))))
        c += RANGE
    return r


def _build(caps, offs, total_slots):
    nc = bacc.Bacc("TRN2", target_bir_lowering=False, debug=False,
                   num_devices=N_CORES, num_swdge_queues=4)

    blocks16 = nc.dram_tensor("blocks16", [N_NODES, D_IN], BF16, kind="ExternalInput")
    gidx_d = nc.dram_tensor("gidx", [P, total_slots // 16], mybir.dt.int16, kind="ExternalInput")
    dstc_d = nc.dram_tensor("dstc", [P, total_slots // P], F32, kind="ExternalInput")
    wv_d = nc.dram_tensor("wv", [P, total_slots // P], F32, kind="ExternalInput")
    iota_d = nc.dram_tensor("iota", [P, CH], F32, kind="ExternalInput")
    convW_d = nc.dram_tensor("convW16", [CONV_LAYERS, N_ETYPES, D_IN, D_IN], BF16, kind="ExternalInput")
    convb_d = nc.dram_tensor("convb3", [CONV_LAYERS, N_ETYPES, D_IN, 1], F32, kind="ExternalInput")
    W12_d = nc.dram_tensor("W12", [D_IN, D_OUT], F32, kind="ExternalInput")
    b12_d = nc.dram_tensor("b12", [D_OUT, 1], F32, kind="ExternalInput")
    y_d = nc.dram_tensor("y", [NPC, D_OUT], F32, kind="ExternalOutput")

    ranges = _ranges()
    qrot = [0]

    with tile.TileContext(nc) as tc:
        with (
            tc.tile_pool(name="const", bufs=1) as cpool,
            tc.tile_pool(name="idx", bufs=6) as idxpool,
            tc.tile_pool(name="msg", bufs=6) as msgpool,
            tc.tile_pool(name="A", bufs=4) as apool,
            tc.tile_pool(name="aggsb", bufs=3) as aggpool,
            tc.tile_pool(name="hacc", bufs=3) as haccpool,
            tc.tile_pool(name="tmp", bufs=3) as tmppool,
            tc.tile_pool(name="stage", bufs=3) as stpool,
            tc.tile_pool(name="dram", bufs=1, space="DRAM") as drampool,
            tc.tile_pool(name="psum_agg", bufs=2, space="PSUM") as ps_agg,
            tc.tile_pool(name="psum_w", bufs=2, space="PSUM") as ps_w,
            tc.tile_pool(name="psum_head", bufs=1, space="PSUM") as ps_head,
            tc.tile_pool(name="psum_t", bufs=1, space="PSUM") as ps_t,
            tc.tile_pool(name="psum_t2", bufs=1, space="PSUM") as ps_t2,
        ):
            h1_bounce = drampool.tile([NPC, D_IN], BF16, name="h1_bounce")
            h1_full = drampool.tile([N_NODES, D_IN], BF16, name="h1_full")
            nc.gpsimd.load_library(mlp)

            dstc_s = cpool.tile([P, total_slots // P], F32)
            nc.sync.dma_start(dstc_s[:], dstc_d[:])
            wv_s = cpool.tile([P, total_slots // P], F32)
            nc.sync.dma_start(wv_s[:], wv_d[:])
            iota_s = cpool.tile([P, CH], F32)
            nc.sync.dma_start(iota_s[:], iota_d[:])
            ident = cpool.tile([P, P], F32)
            make_identity(nc, ident[:])
            Wc = {}
            bc = {}
            for l in range(CONV_LAYERS):
                for e in range(N_ETYPES):
                    Wc[l, e] = cpool.tile([P, P], BF16, name=f"Wc{l}{e}")
                    nc.sync.dma_start(Wc[l, e][:], convW_d[l, e])
                    bc[l, e] = cpool.tile([P, 1], F32, name=f"bc{l}{e}")
                    nc.sync.dma_start(bc[l, e][:], convb_d[l, e])
            W12_s = cpool.tile([P, D_OUT], F32)
            nc.sync.dma_start(W12_s[:], W12_d[:])
            b12_s = cpool.tile([D_OUT, 1], F32)
            nc.sync.dma_start(b12_s[:], b12_d[:])

            def gather_call(src_dram, e, s, chunks):
                n = int(caps[e, s, chunks].sum())
                if n == 0:
                    return None, 0
                off = int(offs[e, s, chunks[0]])
                idx_t = idxpool.tile([P, n // 16], mybir.dt.int16, name="idx_t", tag="idx")
                nc.sync.dma_start(idx_t[:], gidx_d[:, off // 16:(off + n) // 16])
                buf = msgpool.tile([P, n], BF16, name="msgbuf", tag="msg")
                nc.gpsimd.dma_gather(
                    buf[:].rearrange("p (t d) -> p t d", d=D_IN),
                    src_dram[s * SEC:(s + 1) * SEC, :],
                    idx_t[:], n, n, D_IN,
                    single_packet=False, queue_num=qrot[0] % 4,
                )
                qrot[0] += 1
                return buf, off

            def layer(l, src_dram):
                for chunks in ranges:
                    hacc = {}
                    for e in range(N_ETYPES):
                        bufs = {}
                        for s in range(N_SEC):
                            bufs[s] = gather_call(src_dram, e, s, chunks)
                        for c in chunks:
                            pagg = ps_agg.tile([P, CH], F32, name="pagg", tag="pagg")
                            tiles = []
                            for s in range(N_SEC):
                                ntile = int(caps[e, s, c]) // P
                                for t in range(ntile):
                                    tiles.append((s, t))
                            for k, (s, t) in enumerate(tiles):
                                buf, off = bufs[s]
                                # column of this tile within the gather buffer
                                col = (int(offs[e, s, c]) - off) // P + t
                                gt = int(offs[e, s, c]) // P + t   # global tile idx
                                A = apool.tile([P, CH], BF16, name="Atile", tag="A")
                                nc.vector.tensor_scalar(
                                    A[:], iota_s[:],
                                    dstc_s[:, gt:gt + 1], wv_s[:, gt:gt + 1],
                                    mybir.AluOpType.is_equal, mybir.AluOpType.mult,
                                )
                                nc.tensor.matmul(
                                    pagg[:],
                                    lhsT=buf[:, col * P:(col + 1) * P],
                                    rhs=A[:],
                                    start=(k == 0), stop=(k == len(tiles) - 1),
                                )
                            aggT = aggpool.tile([P, CH], BF16, name="aggT", tag="agg")
                            nc.scalar.copy(aggT[:], pagg[:])
                            pw = ps_w.tile([P, CH], F32, name="pw", tag="pw")
                            nc.tensor.matmul(pw[:], lhsT=Wc[l, e][:], rhs=aggT[:],
                                             start=True, stop=True)
                            if e == 0:
                                hacc[c] = haccpool.tile([P, CH], F32, name="hacc", tag="hacc")
                                nc.scalar.activation(
                                    hacc[c][:], pw[:],
                                    mybir.ActivationFunctionType.Relu,
                                    bias=bc[l, e][:, :1])
                            else:
                                tmp = tmppool.tile([P, CH], F32, name="tmpr", tag="tmp")
                                nc.scalar.activation(
                                    tmp[:], pw[:],
                                    mybir.ActivationFunctionType.Relu,
                                    bias=bc[l, e][:, :1])
                                nc.vector.tensor_add(hacc[c][:], hacc[c][:], tmp[:])
                    for c in chunks:
                        nvalid = min(CH, NPC - c * CH)
                        nblk = (nvalid + P - 1) // P
                        if l == 0:
                            pt = ps_t.tile([P, CH], F32, name="pt", tag="pt")
                            for b in range(nblk):
                                nc.tensor.transpose(
                                    pt[:, b * P:(b + 1) * P],
                                    hacc[c][:, b * P:(b + 1) * P], ident[:])
                            st = stpool.tile([P, nblk * P], BF16, name="st", tag="st")
                            nc.vector.tensor_copy(st[:], pt[:, :nblk * P])
                            for b in range(nblk):
                                rows = min(P, nvalid - b * P)
                                nc.sync.dma_start(
                                    h1_bounce[c * CH + b * P:c * CH + b * P + rows, :],
                                    st[:rows, b * P:(b + 1) * P])
                        else:
                            p4 = ps_head.tile([D_OUT, CH], F32, name="p4", tag="p4")
                            nc.tensor.matmul(p4[:], lhsT=W12_s[:], rhs=hacc[c][:],
                                             start=True, stop=True)
                            z = tmppool.tile([D_OUT, CH], F32, name="ztile", tag="z")
                            nc.vector.tensor_scalar(
                                z[:], p4[:], b12_s[:, :1], None,
                                mybir.AluOpType.add)
                            pt = ps_t2.tile([P, nblk * D_OUT], F32, name="pt2", tag="pt2")
                            for b in range(nblk):
                                nc.tensor.transpose(
                                    pt[:, b * D_OUT:(b + 1) * D_OUT],
                                    z[:, b * P:(b + 1) * P], ident[:D_OUT, :D_OUT])
                            st = stpool.tile([P, nblk * D_OUT], F32, name="sty", tag="sty")
                            nc.vector.tensor_copy(st[:], pt[:, :nblk * D_OUT])
                            for b in range(nblk):
                                rows = min(P, nvalid - b * P)
                                nc.sync.dma_start(
                                    y_d[c * CH + b * P:c * CH + b * P + rows, :],
                                    st[:rows, b * D_OUT:(b + 1) * D_OUT])

            layer(0, blocks16)
            nc.gpsimd.collective_compute(
                "AllGather", mybir.AluOpType.bypass,
                replica_groups=[list(range(N_CORES))],
                ins=[h1_bounce.opt()],
                outs=[h1_full.opt()],
            )
            layer(1, h1_full)

    nc.compile()
    return nc


def kernel(blocks, edge_src, edge_dst, conv_W, conv_b, W1, b1, W2, b2):
    blocks = np.asarray(blocks, np.float32)
    edge_src = np.asarray(edge_src, np.int32)
    edge_dst = np.asarray(edge_dst, np.int32)
    conv_W = np.asarray(conv_W, np.float32)
    conv_b = np.asarray(conv_b, np.float32)
    W1 = np.asarray(W1, np.float32)
    b1 = np.asarray(b1, np.float32)
    W2 = np.asarray(W2, np.float32)
    b2 = np.asarray(b2, np.float32)

    in_maps, caps, offs, total_slots = _prep(
        blocks, edge_src, edge_dst, conv_W, conv_b, W1, b1, W2, b2)
    nc = _build(caps, offs, total_slots)
    res = run_bass_kernel_spmd(nc, in_maps, list(range(N_CORES)))
    global LAST_RESULT
    LAST_RESULT = res
    out = np.concatenate([res.results[c]["y"] for c in range(N_CORES)], axis=0)
    return out.astype(np.float32)


LAST_RESULT = None
